# revision 2
# baseline (speedup 1.0000x reference)
"""HashEncoding (multires hash grid, 16 levels, F=2) for 8 trn2 NeuronCores.

Block-gather design: instead of one [128,1]-offset indirect DMA per corner row
(8 bytes useful per 30us instruction -- the old baseline), each level's corner
lookups become int16 block indices (table row >> 5, i.e. 256B-aligned blocks of
32 rows) that are folded into the SWDGE idx-list layout and fetched with
InstDMAGatherAnt: 1024 blocks per instruction, round-robin over the 4 SWDGE
queue core-pairs (queue Q reads its idx list from partitions [32Q, 32Q+32)).
The wanted 8B row is then selected on DVE with an iota==s mask + reduce, and
trilinearly blended.  Per core: 16384 gather instructions instead of 131072
indirect DMAs, ~3.7us each with 4-queue overlap.
"""
import sys

sys.path.insert(0, "/opt/trn_rl_repo")

import numpy as np

NUM_LEVELS = 16
F = 2
TABLE_SIZE = 1 << 19
MASK = TABLE_SIZE - 1
NB = TABLE_SIZE // 32          # 256B blocks per level slice
N_POINTS = 1 << 20
N_CORES = 8
NPC = N_POINTS // N_CORES      # 131072 points per core
SCALINGS = [16.0, 23.0, 33.0, 48.0, 70.0, 101.0, 147.0, 212.0,
            307.0, 445.0, 645.0, 933.0, 1351.0, 1955.0, 2830.0, 4095.0]
P1_19 = 489905      # 2654435761 mod 2^19
P2_19 = 153493      # 805459861  mod 2^19
P1_5 = 17           # 2654435761 mod 32
P2_5 = 21           # 805459861  mod 32

_CACHE = {}


def build_program():
    import concourse.bass as bass
    import concourse.tile as tile
    from concourse import bacc, mybir

    F32 = mybir.dt.float32
    I32 = mybir.dt.int32
    I16 = mybir.dt.int16
    ts = bass.ts

    XOR = mybir.AluOpType.bitwise_xor
    AND = mybir.AluOpType.bitwise_and
    MUL = mybir.AluOpType.mult
    NE = mybir.AluOpType.not_equal
    EQ = mybir.AluOpType.is_equal
    GT = mybir.AluOpType.is_gt
    SHR = mybir.AluOpType.logical_shift_right
    ADD = mybir.AluOpType.add
    AXX = mybir.AxisListType.X

    nc = bacc.Bacc("TRN2", target_bir_lowering=False, debug=False,
                   num_devices=N_CORES, num_swdge_queues=4)
    xs = nc.dram_tensor("xs", [NPC, 3], F32, kind="ExternalInput").ap()
    tbl = nc.dram_tensor("tbl", [TABLE_SIZE * NUM_LEVELS, F], F32,
                         kind="ExternalInput").ap()
    enc = nc.dram_tensor("enc", [NPC, NUM_LEVELS * F], F32,
                         kind="ExternalOutput").ap()

    # table as 256B blocks: [NUM_LEVELS*NB, 64]
    tblb = tbl.rearrange("(b r) f -> b (r f)", r=32)

    # point row r = K*128 + P  (K: global col 0..1023, P: partition)
    xnat_src = xs.rearrange("(k p) d -> p k d", p=128)            # [128,1024,3]
    # virtual layout: partition 16*kk+q holds cols (i, j, d);
    # point(P=16j+q, K=i*8+kk)  ->  r = i*1024 + kk*128 + j*16 + q
    xv_src = xs.rearrange("(i kk j q) d -> kk j q i d",
                          i=128, kk=8, j=8, q=16)           # [8,8,16,128,3]
    # out: for fixed (k, j): rows r = sc*16384 + i*1024 + k*128 + j*16 + q
    enc_dst = enc.rearrange("(sc i k j q) f -> k j q sc i f",
                            sc=8, i=16, k=8, j=8, q=16)  # [8,8,16,8,16,32]

    with tile.TileContext(nc) as tc:
        with (
            tc.tile_pool(name="xp", bufs=1) as xp,
            tc.tile_pool(name="cp", bufs=1) as cp,
            tc.tile_pool(name="hp", bufs=1) as hp,
            tc.tile_pool(name="ibp", bufs=2) as ibp,
            tc.tile_pool(name="itp", bufs=2) as itp,
            tc.tile_pool(name="swp", bufs=1) as swp,
            tc.tile_pool(name="gp", bufs=1) as gp,
            tc.tile_pool(name="selp", bufs=2) as selp,
            tc.tile_pool(name="accp", bufs=2) as accp,
        ):
            xnat = xp.tile([128, 3072], F32)
            nc.sync.dma_start(
                xnat[:].rearrange("p (k d) -> p k d", d=3), xnat_src)
            xv = xp.tile([128, 3072], F32, tag="xv")
            for kk in range(8):
                xvv = xv[16 * kk:16 * (kk + 1), :].rearrange(
                    "p (i j d) -> p i j d", j=8, d=3)
                for j in range(8):
                    nc.sync.dma_start(xvv[:, :, j, :], xv_src[kk][j])

            iot_i = xp.tile([128, 256], I32, tag="ioti")
            nc.gpsimd.iota(iot_i[:], [[0, 8], [1, 32]], channel_multiplier=0)
            iota_f = xp.tile([128, 256], F32, tag="iotf")
            nc.vector.tensor_copy(iota_f[:], iot_i[:])
            iota_v = iota_f[:].rearrange("p (c r) -> p c r", r=32)

            ni_reg = nc.gpsimd.to_reg(1024)

            def hmul(src, p19, tag):
                q1 = (p19 * 32) & MASK
                q2 = (p19 * 1024) & MASK
                y0 = hp.tile([128, 128], I32, tag="hy0")
                nc.vector.tensor_scalar(y0[:], src[:], 31, None, AND)
                y1 = hp.tile([128, 128], I32, tag="hy1")
                nc.vector.tensor_scalar(y1[:], src[:], 5, 31, SHR, AND)
                y2 = hp.tile([128, 128], I32, tag="hy2")
                nc.vector.tensor_scalar(y2[:], src[:], 10, None, SHR)
                m0 = hp.tile([128, 128], I32, tag="hm0")
                nc.vector.tensor_scalar(m0[:], y0[:], p19, None, MUL)
                nc.vector.tensor_scalar(m0[:], m0[:], MASK, None, AND)
                m1 = hp.tile([128, 128], I32, tag="hm1")
                nc.vector.tensor_scalar(m1[:], y1[:], q1, None, MUL)
                nc.vector.tensor_scalar(m1[:], m1[:], MASK, None, AND)
                m2 = hp.tile([128, 128], I32, tag="hm2")
                nc.vector.tensor_scalar(m2[:], y2[:], q2, None, MUL)
                h = hp.tile([128, 128], I32, tag=tag)
                nc.vector.tensor_tensor(h[:], m0[:], m1[:], ADD)
                nc.vector.tensor_tensor(h[:], h[:], m2[:], ADD)
                return h

            # floor/ceil/int decomposition of scaled coord; returns
            # (fi, ci, od): int floor, int ceil, fractional offset (od=None
            # if want_off False)
            def fcp(xt, S, tag, want_off):
                sv = hp.tile([128, 128], F32, tag="fsv")
                nc.vector.tensor_scalar(sv[:], xt, S, None, MUL)
                ri = hp.tile([128, 128], I32, tag="fri")
                nc.vector.tensor_copy(ri[:], sv[:])          # round-nearest
                rf = hp.tile([128, 128], F32, tag="frf")
                nc.vector.tensor_copy(rf[:], ri[:])
                gt = hp.tile([128, 128], F32, tag="fgt")
                nc.vector.tensor_tensor(gt[:], rf[:], sv[:], GT)
                ff = hp.tile([128, 128], F32, tag="fff")
                nc.vector.tensor_sub(ff[:], rf[:], gt[:])    # floor
                ne = hp.tile([128, 128], F32, tag="fne")
                nc.vector.tensor_tensor(ne[:], ff[:], sv[:], NE)
                cf = hp.tile([128, 128], F32, tag="fcf")
                nc.vector.tensor_add(cf[:], ff[:], ne[:])    # ceil
                fi = hp.tile([128, 128], I32, tag=tag + "fi")
                nc.vector.tensor_copy(fi[:], ff[:])
                ci = hp.tile([128, 128], I32, tag=tag + "ci")
                nc.vector.tensor_copy(ci[:], cf[:])
                od = None
                if want_off:
                    od = hp.tile([128, 128], F32, tag=tag + "od")
                    nc.vector.tensor_sub(od[:], sv[:], ff[:])
                return fi, ci, od

            with tc.For_i(0, 8) as sc:
                # per-dim coord tiles for this super-chunk
                xn3 = xnat[:, ts(sc, 384)].rearrange("p (k d) -> p k d", d=3)
                xv3 = xv[:, ts(sc, 384)].rearrange("p (k d) -> p k d", d=3)
                xnd, xvd = [], []
                for d in range(3):
                    t = cp.tile([128, 128], F32, tag=f"xnd{d}")
                    nc.vector.tensor_copy(t[:], xn3[:, :, d])
                    xnd.append(t)
                    t = cp.tile([128, 128], F32, tag=f"xvd{d}")
                    nc.vector.tensor_copy(t[:], xv3[:, :, d])
                    xvd.append(t)

                acc = accp.tile([128, 16, 8, 32], F32, tag="acc")

                for lvl in range(NUM_LEVELS):
                    S = SCALINGS[lvl]

                    # ---- natural pass: block indices IB ----
                    nfi, nci, nod = [], [], []
                    for d in range(3):
                        fi, ci, _ = fcp(xnd[d][:], S, f"d{d}", False)
                        nfi.append(fi)
                        nci.append(ci)
                    h1f = hmul(nfi[1], P1_19, "h1f")
                    h1c = hmul(nci[1], P1_19, "h1c")
                    h2f = hmul(nfi[2], P2_19, "h2f")
                    h2c = hmul(nci[2], P2_19, "h2c")

                    def txor(a, b, tag):
                        t = hp.tile([128, 128], I32, tag=tag)
                        nc.vector.tensor_tensor(t[:], a[:], b[:], XOR)
                        return t

                    t_cc = txor(h1c, h2c, "tcc")
                    t_fc = txor(h1f, h2c, "tfc")
                    t_cf = txor(h1c, h2f, "tcf")
                    t_ff = txor(h1f, h2f, "tff")

                    # corner order per reference CORNER_MASK (1=ceil):
                    # c0=(1,1,1) c1=(1,0,1) c2=(0,0,1) c3=(0,1,1)
                    # c4=(1,1,0) c5=(1,0,0) c6=(0,0,0) c7=(0,1,0)
                    xc, xf = nci[0], nfi[0]
                    cspec = [(xc, t_cc), (xc, t_fc), (xf, t_fc), (xf, t_cc),
                             (xc, t_cf), (xc, t_ff), (xf, t_ff), (xf, t_cf)]
                    ib = ibp.tile([128, 16, 8, 8], I16, tag="ib")  # (i,c,k)
                    for c, (xp_, tp) in enumerate(cspec):
                        raw = hp.tile([128, 128], I32, tag="craw")
                        nc.vector.tensor_tensor(raw[:], xp_[:], tp[:], XOR)
                        bi = hp.tile([128, 128], I32, tag="cbi")
                        nc.vector.tensor_scalar(bi[:], raw[:], MASK, 5,
                                                AND, SHR)
                        nc.vector.tensor_copy(
                            ib[:, :, c, :],
                            bi[:].rearrange("p (i k) -> p i k", k=8))

                    # ---- fold into per-queue idx bands ----
                    # IT[32Q+16t+q, jj*1024 + i*64 + C] = IB[16j+q, i*64+C]
                    it = itp.tile([128, 2048], I16, tag="it")
                    ibf = ib[:].rearrange("p a b c -> p (a b c)")  # [128,1024]
                    for j in range(8):
                        Q, jj = j % 4, j // 4
                        for t in range(2):
                            p0 = 32 * Q + 16 * t
                            nc.sync.dma_start(
                                it[p0:p0 + 16, jj * 1024:(jj + 1) * 1024],
                                ibf[16 * j:16 * (j + 1), :])

                    # ---- virtual pass: s (low 5 bits) and weights ----
                    vfi, vci, vod = [], [], []
                    for d in range(3):
                        fi, ci, od = fcp(xvd[d][:], S, f"d{d}", True)
                        vfi.append(fi)
                        vci.append(ci)
                        vod.append(od)

                    def low5(src, p5, tag):
                        lo = hp.tile([128, 128], I32, tag="llo")
                        nc.vector.tensor_scalar(lo[:], src[:], 31, None, AND)
                        hl = hp.tile([128, 128], I32, tag=tag)
                        nc.vector.tensor_scalar(hl[:], lo[:], p5, None, MUL)
                        nc.vector.tensor_scalar(hl[:], hl[:], 31, None, AND)
                        return hl

                    l1f = low5(vfi[1], P1_5, "l1f")
                    l1c = low5(vci[1], P1_5, "l1c")
                    l2f = low5(vfi[2], P2_5, "l2f")
                    l2c = low5(vci[2], P2_5, "l2c")
                    tl_cc = txor(l1c, l2c, "tcc")
                    tl_fc = txor(l1f, l2c, "tfc")
                    tl_cf = txor(l1c, l2f, "tcf")
                    tl_ff = txor(l1f, l2f, "tff")
                    xlf = hp.tile([128, 128], I32, tag="xlf")
                    nc.vector.tensor_scalar(xlf[:], vfi[0][:], 31, None, AND)
                    xlc = hp.tile([128, 128], I32, tag="xlc")
                    nc.vector.tensor_scalar(xlc[:], vci[0][:], 31, None, AND)

                    st = swp.tile([128, 16, 8, 8], F32, tag="st")  # (i,j,c)
                    lspec = [(xlc, tl_cc), (xlc, tl_fc), (xlf, tl_fc),
                             (xlf, tl_cc), (xlc, tl_cf), (xlc, tl_ff),
                             (xlf, tl_ff), (xlf, tl_cf)]
                    for c, (xp_, tp) in enumerate(lspec):
                        sraw = hp.tile([128, 128], I32, tag="craw")
                        nc.vector.tensor_tensor(sraw[:], xp_[:], tp[:], XOR)
                        nc.vector.tensor_copy(
                            st[:, :, :, c],
                            sraw[:].rearrange("p (i j) -> p i j", j=8))

                    ox, oy, oz = vod

                    def onem(o, tag):
                        t = hp.tile([128, 128], F32, tag=tag)
                        nc.vector.tensor_scalar(t[:], o[:], -1.0, 1.0, MUL, ADD)
                        return t

                    bxx = onem(ox, "bxx")
                    byy = onem(oy, "byy")
                    bzz = onem(oz, "bzz")

                    def tmul(a, b, tag):
                        t = hp.tile([128, 128], F32, tag=tag)
                        nc.vector.tensor_mul(t[:], a[:], b[:])
                        return t

                    u_cc = tmul(oy, oz, "ucc")
                    u_fc = tmul(byy, oz, "ufc")
                    u_cf = tmul(oy, bzz, "ucf")
                    u_ff = tmul(byy, bzz, "uff")
                    wt = swp.tile([128, 16, 8, 8], F32, tag="wt")  # (i,j,c)
                    wspec = [(ox, u_cc), (ox, u_fc), (bxx, u_fc), (bxx, u_cc),
                             (ox, u_cf), (ox, u_ff), (bxx, u_ff), (bxx, u_cf)]
                    for c, (a, b) in enumerate(wspec):
                        nc.vector.tensor_tensor(
                            wt[:, :, :, c], a[:].rearrange("p (i j) -> p i j", j=8),
                            b[:].rearrange("p (i j) -> p i j", j=8), MUL)

                    # ---- gathers + select + blend ----
                    tsl = tblb[lvl * NB:(lvl + 1) * NB, :]
                    with tc.For_i(0, 16) as il:
                        for j in range(8):
                            jj = j // 4
                            idxs = it[:, jj * 1024:(jj + 1) * 1024][:, ts(il, 64)]
                            gt_ = gp.tile([128, 8, 64], F32, tag=f"g{j}")
                            nc.gpsimd.dma_gather(gt_[:], tsl, idxs, 1024,
                                                 ni_reg, 64, queue_num=j % 4)
                            s_sl = st[:, ts(il, 1), j, :].rearrange(
                                "p a c -> p (a c)")
                            m = selp.tile([128, 8, 32], F32, tag="m")
                            nc.vector.tensor_tensor(
                                m[:], iota_v, s_sl.to_broadcast([128, 8, 32]),
                                EQ)
                            pr = selp.tile([128, 8, 32, 2], F32, tag="pr")
                            gv = gt_[:].rearrange("p c (r f) -> p c r f", f=2)
                            nc.vector.tensor_tensor(
                                pr[:], gv, m[:].to_broadcast([128, 8, 32, 2]),
                                MUL)
                            fsel = selp.tile([128, 8, 2], F32, tag="fs")
                            nc.vector.tensor_reduce(
                                fsel[:], pr[:].rearrange("p c r f -> p c f r"),
                                AXX, ADD)
                            w_sl = wt[:, ts(il, 1), j, :].rearrange(
                                "p a c -> p (a c)")
                            wf = selp.tile([128, 8, 2], F32, tag="wf")
                            nc.vector.tensor_tensor(
                                wf[:], fsel[:],
                                w_sl.to_broadcast([128, 8, 2]), MUL)
                            nc.vector.tensor_reduce(
                                acc[:, ts(il, 1), j, lvl * F:(lvl + 1) * F],
                                wf[:].rearrange("p c f -> p f c"), AXX, ADD)

                # ---- write out this super-chunk ----
                for k in range(8):
                    for j in range(8):
                        src = acc[16 * k:16 * (k + 1), :, j, :]
                        dst = enc_dst[k][j][:, ts(sc, 1)].rearrange(
                            "q s i f -> q (s i) f")
                        nc.sync.dma_start(dst, src)
    nc.compile()
    return nc


def _get_program():
    if "nc" not in _CACHE:
        _CACHE["nc"] = build_program()
    return _CACHE["nc"]


def kernel(x: np.ndarray, hash_table: np.ndarray) -> np.ndarray:
    from concourse.bass_utils import run_bass_kernel_spmd

    nc = _get_program()
    x = np.ascontiguousarray(np.asarray(x, dtype=np.float32))
    tb = np.ascontiguousarray(np.asarray(hash_table, dtype=np.float32))
    in_maps = [
        {"xs": x[c * NPC:(c + 1) * NPC], "tbl": tb} for c in range(N_CORES)
    ]
    res = run_bass_kernel_spmd(nc, in_maps, list(range(N_CORES)))
    return np.concatenate(
        [res.results[c]["enc"] for c in range(N_CORES)], axis=0)


# revision 3
# speedup vs baseline: 1.0289x; 1.0289x over previous
"""HashEncoding (multires hash grid, 16 levels, F=2) for 8 trn2 NeuronCores.

Block-gather design: instead of one [128,1]-offset indirect DMA per corner row
(8 bytes useful per 30us instruction -- the old baseline), each level's corner
lookups become int16 block indices (table row >> 5, i.e. 256B-aligned blocks of
32 rows) that are folded into the SWDGE idx-list layout and fetched with
InstDMAGatherAnt: 1024 blocks per instruction, round-robin over the 4 SWDGE
queue core-pairs (queue Q reads its idx list from partitions [32Q, 32Q+32)).
The wanted 8B row is then selected on DVE with an iota==s mask + reduce, and
trilinearly blended.  Per core: 16384 gather instructions instead of 131072
indirect DMAs, ~3.7us each with 4-queue overlap.
"""
import sys

sys.path.insert(0, "/opt/trn_rl_repo")

import numpy as np

NUM_LEVELS = 16
F = 2
TABLE_SIZE = 1 << 19
MASK = TABLE_SIZE - 1
NB = TABLE_SIZE // 32          # 256B blocks per level slice
N_POINTS = 1 << 20
N_CORES = 8
NPC = N_POINTS // N_CORES      # 131072 points per core
SCALINGS = [16.0, 23.0, 33.0, 48.0, 70.0, 101.0, 147.0, 212.0,
            307.0, 445.0, 645.0, 933.0, 1351.0, 1955.0, 2830.0, 4095.0]
P1_19 = 489905      # 2654435761 mod 2^19
P2_19 = 153493      # 805459861  mod 2^19
P1_5 = 17           # 2654435761 mod 32
P2_5 = 21           # 805459861  mod 32

_CACHE = {}


def build_program():
    import concourse.bass as bass
    import concourse.tile as tile
    from concourse import bacc, mybir

    F32 = mybir.dt.float32
    I32 = mybir.dt.int32
    I16 = mybir.dt.int16
    ts = bass.ts

    XOR = mybir.AluOpType.bitwise_xor
    AND = mybir.AluOpType.bitwise_and
    MUL = mybir.AluOpType.mult
    NE = mybir.AluOpType.not_equal
    EQ = mybir.AluOpType.is_equal
    GT = mybir.AluOpType.is_gt
    SHR = mybir.AluOpType.logical_shift_right
    ADD = mybir.AluOpType.add
    AXX = mybir.AxisListType.X

    nc = bacc.Bacc("TRN2", target_bir_lowering=False, debug=False,
                   num_devices=N_CORES, num_swdge_queues=4)
    xs = nc.dram_tensor("xs", [NPC, 3], F32, kind="ExternalInput").ap()
    tbl = nc.dram_tensor("tbl", [TABLE_SIZE * NUM_LEVELS, F], F32,
                         kind="ExternalInput").ap()
    enc = nc.dram_tensor("enc", [NPC, NUM_LEVELS * F], F32,
                         kind="ExternalOutput").ap()

    # table as 256B blocks: [NUM_LEVELS*NB, 64]
    tblb = tbl.rearrange("(b r) f -> b (r f)", r=32)

    # point row r = K*128 + P  (K: global col 0..1023, P: partition)
    xnat_src = xs.rearrange("(k p) d -> p k d", p=128)            # [128,1024,3]
    # virtual layout: partition 16*kk+q holds cols (i, j, d);
    # point(P=16j+q, K=i*8+kk)  ->  r = i*1024 + kk*128 + j*16 + q
    xv_src = xs.rearrange("(i kk j q) d -> kk j q i d",
                          i=128, kk=8, j=8, q=16)           # [8,8,16,128,3]
    # out: for fixed (k, j): rows r = sc*16384 + i*1024 + k*128 + j*16 + q
    enc_dst = enc.rearrange("(sc i k j q) f -> k j q sc i f",
                            sc=8, i=16, k=8, j=8, q=16)  # [8,8,16,8,16,32]

    with tile.TileContext(nc) as tc:
        with (
            tc.tile_pool(name="xp", bufs=1) as xp,
            tc.tile_pool(name="cp", bufs=1) as cp,
            tc.tile_pool(name="hp", bufs=1) as hp,
            tc.tile_pool(name="ibp", bufs=2) as ibp,
            tc.tile_pool(name="itp", bufs=2) as itp,
            tc.tile_pool(name="swp", bufs=1) as swp,
            tc.tile_pool(name="gp", bufs=2) as gp,
            tc.tile_pool(name="selp", bufs=2) as selp,
            tc.tile_pool(name="accp", bufs=2) as accp,
        ):
            xnat = xp.tile([128, 3072], F32)
            nc.sync.dma_start(
                xnat[:].rearrange("p (k d) -> p k d", d=3), xnat_src)
            xv = xp.tile([128, 3072], F32, tag="xv")
            for kk in range(8):
                xvv = xv[16 * kk:16 * (kk + 1), :].rearrange(
                    "p (i j d) -> p i j d", j=8, d=3)
                for j in range(8):
                    nc.sync.dma_start(xvv[:, :, j, :], xv_src[kk][j])

            iot_i = xp.tile([128, 256], I32, tag="ioti")
            nc.gpsimd.iota(iot_i[:], [[0, 8], [1, 32]], channel_multiplier=0)
            iota_f = xp.tile([128, 256], F32, tag="iotf")
            nc.vector.tensor_copy(iota_f[:], iot_i[:])
            iota_v = iota_f[:].rearrange("p (c r) -> p c r", r=32)

            ni_reg = nc.gpsimd.to_reg(1024)

            def hmul(src, p19, tag):
                q1 = (p19 * 32) & MASK
                q2 = (p19 * 1024) & MASK
                y0 = hp.tile([128, 128], I32, tag="hy0")
                nc.vector.tensor_scalar(y0[:], src[:], 31, None, AND)
                y1 = hp.tile([128, 128], I32, tag="hy1")
                nc.vector.tensor_scalar(y1[:], src[:], 5, 31, SHR, AND)
                y2 = hp.tile([128, 128], I32, tag="hy2")
                nc.vector.tensor_scalar(y2[:], src[:], 10, None, SHR)
                m0 = hp.tile([128, 128], I32, tag="hm0")
                nc.vector.tensor_scalar(m0[:], y0[:], p19, None, MUL)
                nc.vector.tensor_scalar(m0[:], m0[:], MASK, None, AND)
                m1 = hp.tile([128, 128], I32, tag="hm1")
                nc.vector.tensor_scalar(m1[:], y1[:], q1, None, MUL)
                nc.vector.tensor_scalar(m1[:], m1[:], MASK, None, AND)
                m2 = hp.tile([128, 128], I32, tag="hm2")
                nc.vector.tensor_scalar(m2[:], y2[:], q2, None, MUL)
                h = hp.tile([128, 128], I32, tag=tag)
                nc.vector.tensor_tensor(h[:], m0[:], m1[:], ADD)
                nc.vector.tensor_tensor(h[:], h[:], m2[:], ADD)
                return h

            # floor/ceil/int decomposition of scaled coord; returns
            # (fi, ci, od): int floor, int ceil, fractional offset (od=None
            # if want_off False)
            def fcp(xt, S, tag, want_off):
                sv = hp.tile([128, 128], F32, tag="fsv")
                nc.vector.tensor_scalar(sv[:], xt, S, None, MUL)
                ri = hp.tile([128, 128], I32, tag="fri")
                nc.vector.tensor_copy(ri[:], sv[:])          # round-nearest
                rf = hp.tile([128, 128], F32, tag="frf")
                nc.vector.tensor_copy(rf[:], ri[:])
                gt = hp.tile([128, 128], F32, tag="fgt")
                nc.vector.tensor_tensor(gt[:], rf[:], sv[:], GT)
                ff = hp.tile([128, 128], F32, tag="fff")
                nc.vector.tensor_sub(ff[:], rf[:], gt[:])    # floor
                ne = hp.tile([128, 128], F32, tag="fne")
                nc.vector.tensor_tensor(ne[:], ff[:], sv[:], NE)
                cf = hp.tile([128, 128], F32, tag="fcf")
                nc.vector.tensor_add(cf[:], ff[:], ne[:])    # ceil
                fi = hp.tile([128, 128], I32, tag=tag + "fi")
                nc.vector.tensor_copy(fi[:], ff[:])
                ci = hp.tile([128, 128], I32, tag=tag + "ci")
                nc.vector.tensor_copy(ci[:], cf[:])
                od = None
                if want_off:
                    od = hp.tile([128, 128], F32, tag=tag + "od")
                    nc.vector.tensor_sub(od[:], sv[:], ff[:])
                return fi, ci, od

            with tc.For_i(0, 8) as sc:
                # per-dim coord tiles for this super-chunk
                xn3 = xnat[:, ts(sc, 384)].rearrange("p (k d) -> p k d", d=3)
                xv3 = xv[:, ts(sc, 384)].rearrange("p (k d) -> p k d", d=3)
                xnd, xvd = [], []
                for d in range(3):
                    t = cp.tile([128, 128], F32, tag=f"xnd{d}")
                    nc.vector.tensor_copy(t[:], xn3[:, :, d])
                    xnd.append(t)
                    t = cp.tile([128, 128], F32, tag=f"xvd{d}")
                    nc.vector.tensor_copy(t[:], xv3[:, :, d])
                    xvd.append(t)

                acc = accp.tile([128, 16, 8, 32], F32, tag="acc")

                for lvl in range(NUM_LEVELS):
                    S = SCALINGS[lvl]

                    # ---- natural pass: block indices IB ----
                    nfi, nci, nod = [], [], []
                    for d in range(3):
                        fi, ci, _ = fcp(xnd[d][:], S, f"d{d}", False)
                        nfi.append(fi)
                        nci.append(ci)
                    h1f = hmul(nfi[1], P1_19, "h1f")
                    h1c = hmul(nci[1], P1_19, "h1c")
                    h2f = hmul(nfi[2], P2_19, "h2f")
                    h2c = hmul(nci[2], P2_19, "h2c")

                    def txor(a, b, tag):
                        t = hp.tile([128, 128], I32, tag=tag)
                        nc.vector.tensor_tensor(t[:], a[:], b[:], XOR)
                        return t

                    t_cc = txor(h1c, h2c, "tcc")
                    t_fc = txor(h1f, h2c, "tfc")
                    t_cf = txor(h1c, h2f, "tcf")
                    t_ff = txor(h1f, h2f, "tff")

                    # corner order per reference CORNER_MASK (1=ceil):
                    # c0=(1,1,1) c1=(1,0,1) c2=(0,0,1) c3=(0,1,1)
                    # c4=(1,1,0) c5=(1,0,0) c6=(0,0,0) c7=(0,1,0)
                    xc, xf = nci[0], nfi[0]
                    cspec = [(xc, t_cc), (xc, t_fc), (xf, t_fc), (xf, t_cc),
                             (xc, t_cf), (xc, t_ff), (xf, t_ff), (xf, t_cf)]
                    ib = ibp.tile([128, 16, 8, 8], I16, tag="ib")  # (i,c,k)
                    for c, (xp_, tp) in enumerate(cspec):
                        raw = hp.tile([128, 128], I32, tag="craw")
                        nc.vector.tensor_tensor(raw[:], xp_[:], tp[:], XOR)
                        bi = hp.tile([128, 128], I32, tag="cbi")
                        nc.vector.tensor_scalar(bi[:], raw[:], MASK, 5,
                                                AND, SHR)
                        nc.vector.tensor_copy(
                            ib[:, :, c, :],
                            bi[:].rearrange("p (i k) -> p i k", k=8))

                    # ---- fold into per-queue idx bands ----
                    # IT[32Q+16t+q, jj*1024 + i*64 + C] = IB[16j+q, i*64+C]
                    it = itp.tile([128, 2048], I16, tag="it")
                    ibf = ib[:].rearrange("p a b c -> p (a b c)")  # [128,1024]
                    for j in range(8):
                        Q, jj = j % 4, j // 4
                        for t in range(2):
                            p0 = 32 * Q + 16 * t
                            nc.sync.dma_start(
                                it[p0:p0 + 16, jj * 1024:(jj + 1) * 1024],
                                ibf[16 * j:16 * (j + 1), :])

                    # ---- virtual pass: s (low 5 bits) and weights ----
                    vfi, vci, vod = [], [], []
                    for d in range(3):
                        fi, ci, od = fcp(xvd[d][:], S, f"d{d}", True)
                        vfi.append(fi)
                        vci.append(ci)
                        vod.append(od)

                    def low5(src, p5, tag):
                        lo = hp.tile([128, 128], I32, tag="llo")
                        nc.vector.tensor_scalar(lo[:], src[:], 31, None, AND)
                        hl = hp.tile([128, 128], I32, tag=tag)
                        nc.vector.tensor_scalar(hl[:], lo[:], p5, None, MUL)
                        nc.vector.tensor_scalar(hl[:], hl[:], 31, None, AND)
                        return hl

                    l1f = low5(vfi[1], P1_5, "l1f")
                    l1c = low5(vci[1], P1_5, "l1c")
                    l2f = low5(vfi[2], P2_5, "l2f")
                    l2c = low5(vci[2], P2_5, "l2c")
                    tl_cc = txor(l1c, l2c, "tcc")
                    tl_fc = txor(l1f, l2c, "tfc")
                    tl_cf = txor(l1c, l2f, "tcf")
                    tl_ff = txor(l1f, l2f, "tff")
                    xlf = hp.tile([128, 128], I32, tag="xlf")
                    nc.vector.tensor_scalar(xlf[:], vfi[0][:], 31, None, AND)
                    xlc = hp.tile([128, 128], I32, tag="xlc")
                    nc.vector.tensor_scalar(xlc[:], vci[0][:], 31, None, AND)

                    st = swp.tile([128, 16, 8, 8], F32, tag="st")  # (i,j,c)
                    lspec = [(xlc, tl_cc), (xlc, tl_fc), (xlf, tl_fc),
                             (xlf, tl_cc), (xlc, tl_cf), (xlc, tl_ff),
                             (xlf, tl_ff), (xlf, tl_cf)]
                    for c, (xp_, tp) in enumerate(lspec):
                        sraw = hp.tile([128, 128], I32, tag="craw")
                        nc.vector.tensor_tensor(sraw[:], xp_[:], tp[:], XOR)
                        nc.vector.tensor_copy(
                            st[:, :, :, c],
                            sraw[:].rearrange("p (i j) -> p i j", j=8))

                    ox, oy, oz = vod

                    def onem(o, tag):
                        t = hp.tile([128, 128], F32, tag=tag)
                        nc.vector.tensor_scalar(t[:], o[:], -1.0, 1.0, MUL, ADD)
                        return t

                    bxx = onem(ox, "bxx")
                    byy = onem(oy, "byy")
                    bzz = onem(oz, "bzz")

                    def tmul(a, b, tag):
                        t = hp.tile([128, 128], F32, tag=tag)
                        nc.vector.tensor_mul(t[:], a[:], b[:])
                        return t

                    u_cc = tmul(oy, oz, "ucc")
                    u_fc = tmul(byy, oz, "ufc")
                    u_cf = tmul(oy, bzz, "ucf")
                    u_ff = tmul(byy, bzz, "uff")
                    wt = swp.tile([128, 16, 8, 8], F32, tag="wt")  # (i,j,c)
                    wspec = [(ox, u_cc), (ox, u_fc), (bxx, u_fc), (bxx, u_cc),
                             (ox, u_cf), (ox, u_ff), (bxx, u_ff), (bxx, u_cf)]
                    for c, (a, b) in enumerate(wspec):
                        nc.vector.tensor_tensor(
                            wt[:, :, :, c], a[:].rearrange("p (i j) -> p i j", j=8),
                            b[:].rearrange("p (i j) -> p i j", j=8), MUL)

                    # ---- gathers + select + blend ----
                    tsl = tblb[lvl * NB:(lvl + 1) * NB, :]
                    with tc.For_i(0, 16) as il:
                        gts = []
                        for j in range(8):
                            jj = j // 4
                            idxs = it[:, jj * 1024:(jj + 1) * 1024][:, ts(il, 64)]
                            gt_ = gp.tile([128, 8, 64], F32, tag=f"g{j}")
                            nc.gpsimd.dma_gather(gt_[:], tsl, idxs, 1024,
                                                 ni_reg, 64, queue_num=j % 4)
                            gts.append(gt_)
                        for j in range(8):
                            gt_ = gts[j]
                            s_sl = st[:, ts(il, 1), j, :].rearrange(
                                "p a c -> p (a c)")
                            m = selp.tile([128, 8, 32], F32, tag="m")
                            nc.vector.tensor_tensor(
                                m[:], iota_v, s_sl.to_broadcast([128, 8, 32]),
                                EQ)
                            pr = selp.tile([128, 8, 32, 2], F32, tag="pr")
                            gv = gt_[:].rearrange("p c (r f) -> p c r f", f=2)
                            nc.vector.tensor_tensor(
                                pr[:], gv, m[:].to_broadcast([128, 8, 32, 2]),
                                MUL)
                            fsel = selp.tile([128, 8, 2], F32, tag="fs")
                            nc.vector.tensor_reduce(
                                fsel[:], pr[:].rearrange("p c r f -> p c f r"),
                                AXX, ADD)
                            w_sl = wt[:, ts(il, 1), j, :].rearrange(
                                "p a c -> p (a c)")
                            wf = selp.tile([128, 8, 2], F32, tag="wf")
                            nc.vector.tensor_tensor(
                                wf[:], fsel[:],
                                w_sl.to_broadcast([128, 8, 2]), MUL)
                            nc.vector.tensor_reduce(
                                acc[:, ts(il, 1), j, lvl * F:(lvl + 1) * F],
                                wf[:].rearrange("p c f -> p f c"), AXX, ADD)

                # ---- write out this super-chunk ----
                for k in range(8):
                    for j in range(8):
                        src = acc[16 * k:16 * (k + 1), :, j, :]
                        dst = enc_dst[k][j][:, ts(sc, 1)].rearrange(
                            "q s i f -> q (s i) f")
                        nc.sync.dma_start(dst, src)
    nc.compile()
    return nc


def _get_program():
    if "nc" not in _CACHE:
        _CACHE["nc"] = build_program()
    return _CACHE["nc"]


def kernel(x: np.ndarray, hash_table: np.ndarray) -> np.ndarray:
    from concourse.bass_utils import run_bass_kernel_spmd

    nc = _get_program()
    x = np.ascontiguousarray(np.asarray(x, dtype=np.float32))
    tb = np.ascontiguousarray(np.asarray(hash_table, dtype=np.float32))
    in_maps = [
        {"xs": x[c * NPC:(c + 1) * NPC], "tbl": tb} for c in range(N_CORES)
    ]
    res = run_bass_kernel_spmd(nc, in_maps, list(range(N_CORES)))
    return np.concatenate(
        [res.results[c]["enc"] for c in range(N_CORES)], axis=0)
